# revision 1
# baseline (speedup 1.0000x reference)
"""Trainium2 Bass kernel for nn_MetaFunUpdaterLocal (gnn_message_passing).

Math (per meta-batch b, per outer-tile k):
    h    = concat([x[b], y[b], r_c[b,k]], -1)           [C, 160]
    U    = MLP(h)  (160->128 relu ->128 relu ->64)      [C, 64]
    next_r_c[b,k] = r_c[b,k] - 0.1 * c_att[b] @ U
    next_r_q[b,k] = r_q[b,k] - 0.1 * q_att[b] @ U

Key restructurings:
  * The x/y part of layer 1 is k-independent: P[b] = [x|y]@W1[:96] + b1 is
    precomputed on host and injected into PSUM with an exact identity matmul.
  * Everything on-device runs in "feature-major" (transposed) space: the host
    ships rT[b,g] = [128, 512] tiles holding the TRANSPOSED r_c / r_q of two
    consecutive pairs (g = pair group), so the same SBUF tile is both the
    layer-1 matmul rhs (contraction dim d on partitions) and the update-term
    operand, with fully contiguous 256 KiB DMAs in and out.
  * All big matmuls run as float32r (full PE rate for moving dim >= 256,
    fp32 storage); only the tiny layer-3 (K=128 -> 64) runs in bf16.
  * delta_c / delta_q for the two pairs are computed with pair-packed
    stationary operands (M = 2x64 = 128) so the PE array is fully used, and
    the attention maps are pre-scaled by -0.1 so the final update is a single
    [128, 512] tensor_tensor add straight out of PSUM.

Sharding: 8 cores, core c handles b = c//2 and a 64-group (128-pair) slice of
the outer C axis (matches the sharding hint: B x outer-C data parallel).
"""

import numpy as np

B, C, Q, XD, YD, E, H = 4, 256, 256, 64, 32, 64, 128
NCORES = 8
NG_CORE = 64  # 2-pair groups per core (64 groups x 2 pairs x 8 cores = 1024 pairs)

_NC_CACHE = {}


def _build_nc(ng=NG_CORE, stage=5):
    import concourse.bass as bass
    import concourse.bacc as bacc
    import concourse.mybir as mybir
    import concourse.tile as tile
    from concourse.bass import _add_dep_helper

    F32 = mybir.dt.float32
    F32R = mybir.dt.float32r
    BF16 = mybir.dt.bfloat16
    ADD = mybir.AluOpType.add
    MAX = mybir.AluOpType.max
    RELU = mybir.ActivationFunctionType.Relu
    COPY = mybir.ActivationFunctionType.Copy

    # Bacc (not raw Bass): its finalize() runs move_matmul_waits_to_ldweights
    # + generate_event_semaphores, which split multi-sem waits to satisfy the
    # TRN2 one-wait-per-instruction constraint.
    nc = bacc.Bacc("TRN2", target_bir_lowering=False, debug=False,
                   num_devices=NCORES)

    # all fp32 constants ship in ONE dma (single wait lane for consumers):
    # cols: [0:512 pt2][512:1024 ac][1024:1536 aq][1536:1664 i128]
    #       [1664:1792 w1d2][1792:1920 w2][1920:1984 w3(f32)]
    #       row0: [1984:2112 ones][2112:2368 b3r]
    CB = 2368
    rt_d = nc.dram_tensor("rt", [ng, 128, 512], F32R, kind="ExternalInput")
    cb_d = nc.dram_tensor("cbig", [128, CB], F32R, kind="ExternalInput")
    out_d = nc.dram_tensor("out", [ng, 128, 512], F32, kind="ExternalOutput")

    with tile.TileContext(nc) as tc:
        with (
            tc.tile_pool(name="const", bufs=1) as cp,
            tc.tile_pool(name="rt", bufs=4) as rtp,
            tc.tile_pool(name="rtr", bufs=3) as rp2,
            tc.tile_pool(name="s", bufs=4) as sp,
            tc.tile_pool(name="u", bufs=3) as up,
            tc.tile_pool(name="o", bufs=4) as op,
            tc.tile_pool(name="pz", bufs=4, space="PSUM") as pz,
            tc.tile_pool(name="pu", bufs=2, space="PSUM") as pu,
            tc.tile_pool(name="pd", bufs=2, space="PSUM") as pd,
        ):
            cbig = cp.tile([128, CB], F32R)
            nc.sync.dma_start(cbig[:], cb_d[:, :])
            pt2 = cbig[:, 0:512]
            ac = cbig[:, 512:1024]
            aq = cbig[:, 1024:1536]
            i128 = cbig[:, 1536:1664]
            w1d2 = cbig[:, 1664:1792]
            w2 = cbig[:, 1792:1920]
            ones1 = cbig[0:1, 1984:2112]
            b3r = cbig[0:1, 2112:2368]
            # w3 via DVE cast-copy so layer-3 matmul deps stay on the DVE sem
            w3 = cp.tile([128, 64], BF16)
            nc.vector.tensor_copy(w3[:], cbig[:, 1920:1984].bitcast(F32))

            def chain(mms):
                for a, b_ in zip(mms[1:], mms):
                    _add_dep_helper(a.ins, b_.ins, sync=False, reason="psum order")

            for g in range(ng):
                rt = rtp.tile([128, 512], F32R)
                ld = nc.sync.dma_start(rt[:], rt_d[g, :, :])
                # DVE nop that owns the rt-DMA wait: the HW allows only ONE
                # sync-wait per compute instruction, so the final update op
                # must not need both the PE (dp) and DMA (rt) waits itself.
                nop = nc.vector.engine_nop()
                _add_dep_helper(nop.ins, ld.ins, sync=True,
                                reason="absorb rt dma wait on DVE")
                # rcT of both pairs side-by-side at base partition 0 (matmul
                # operands at base_partition 64 fault on HW): SBUF->SBUF remap
                rtr = rp2.tile([64, 512], F32R)
                nc.sync.dma_start(rtr[:, 0:256], rt[0:64, 0:256])
                nc.sync.dma_start(rtr[:, 256:512], rt[64:128, 0:256])

                o2 = op.tile([128, 512], F32)
                if stage == 0:
                    nc.vector.tensor_tensor(o2[:], rt[:].bitcast(F32),
                                            rt[:].bitcast(F32), op=ADD)
                    nc.sync.dma_start(out_d[g, :, :], o2[:])
                    continue

                # ---- layer 1: Z1[h, i] = P[b].T (+) W1d.T @ rcT, per pair
                z1 = pz.tile([128, 512], F32, tag="z")
                if stage == 11:  # identity matmul only (f32r)
                    nc.tensor.matmul(z1[:], i128[:], pt2[:], start=True, stop=True)
                else:
                    m0 = nc.tensor.matmul(z1[:], i128[:], pt2[:],
                                          start=True, stop=False)
                    m1 = nc.tensor.matmul(z1[:], w1d2[0:64, :], rtr[:],
                                          start=False, stop=True)
                    chain([m0, m1])
                s1 = sp.tile([128, 512], F32R, tag="s1")
                nc.scalar.activation(s1[:], z1[:], RELU)
                if stage in (1, 11):
                    nc.vector.tensor_copy(o2[:], s1[:].bitcast(F32))
                    nc.sync.dma_start(out_d[g, :, :], o2[:])
                    continue

                # ---- layer 2
                z2 = pz.tile([128, 512], F32, tag="z")
                nc.tensor.matmul(z2[:], w2[:], s1[:], start=True, stop=True)
                s2 = sp.tile([128, 512], BF16, tag="s2")
                nc.vector.tensor_scalar_max(s2[:], z2[:], 0.0)
                if stage == 2:
                    nc.vector.tensor_copy(o2[:], s2[:])
                    nc.sync.dma_start(out_d[g, :, :], o2[:])
                    continue

                # ---- layer 3: U[j, e] per (pair, j-chunk), +b3 via K=1 matmul
                # ups columns: [A-ch0 | B-ch0 | A-ch1 | B-ch1], 64 each
                ups = pu.tile([128, 256], F32)
                mb3 = nc.tensor.matmul(ups[:], ones1[:, :], b3r[:, :],
                                       start=True, stop=False)
                umms = [mb3]
                for ch in range(2):
                    for p in range(2):
                        mm = nc.tensor.matmul(
                            ups[:, ch * 128 + p * 64: ch * 128 + p * 64 + 64],
                            s2[:, p * 256 + ch * 128: p * 256 + (ch + 1) * 128],
                            w3[:],
                            start=False, stop=(ch == 1 and p == 1))
                        umms.append(mm)
                chain(umms)
                u = up.tile([128, 256], F32R)
                nc.scalar.activation(u[:], ups[:], COPY)
                if stage == 3:
                    nc.vector.tensor_copy(o2[:, 0:256], u[:].bitcast(F32))
                    nc.vector.tensor_copy(o2[:, 256:512], u[:].bitcast(F32))
                    nc.sync.dma_start(out_d[g, :, :], o2[:])
                    continue

                # ---- deltas: pd = [-0.1*Ac@U | -0.1*Aq@U], pair-packed M=128
                dp = pd.tile([128, 512], F32)
                d0 = nc.tensor.matmul(dp[:, 0:256], u[:, 0:128],
                                      ac[:, 0:256], start=True, stop=False)
                d1 = nc.tensor.matmul(dp[:, 0:256], u[:, 128:256],
                                      ac[:, 256:512], start=False, stop=False)
                d2 = nc.tensor.matmul(dp[:, 256:512], u[:, 0:128],
                                      aq[:, 0:256], start=False, stop=False)
                d3 = nc.tensor.matmul(dp[:, 256:512], u[:, 128:256],
                                      aq[:, 256:512], start=False, stop=True)
                chain([d0, d1, d2, d3])
                if stage == 4:
                    nc.vector.tensor_copy(o2[:], dp[:])
                    nc.sync.dma_start(out_d[g, :, :], o2[:])
                    continue

                # ---- update + store
                # first-writer memset absorbs the o2 slot-release (store DMA)
                # wait so the update op itself only waits on PE
                nc.vector.memset(o2[0:1, 0:1], 0.0)
                nc.vector.tensor_tensor(o2[:], rt[:].bitcast(F32), dp[:], op=ADD)
                nc.sync.dma_start(out_d[g, :, :], o2[:])

    nc.finalize()
    return nc


def _get_nc(ng=NG_CORE):
    if ng not in _NC_CACHE:
        _NC_CACHE[ng] = _build_nc(ng)
    return _NC_CACHE[ng]


def _host_prep(x, y, r_c, r_q, c_att_map, q_att_map, W1, b1, W2, b2, W3, b3):
    """Build per-core input maps. Returns (in_maps, meta)."""
    import ml_dtypes

    f32 = np.float32
    x = np.asarray(x, f32); y = np.asarray(y, f32)
    r_c = np.ascontiguousarray(np.asarray(r_c, f32))
    r_q = np.ascontiguousarray(np.asarray(r_q, f32))
    c_att = np.asarray(c_att_map, f32); q_att = np.asarray(q_att_map, f32)
    W1 = np.asarray(W1, f32); b1 = np.asarray(b1, f32)
    W2 = np.asarray(W2, f32); W3 = np.asarray(W3, f32); b3 = np.asarray(b3, f32)

    # P[b] = [x|y] @ W1[:96] + b1  (k-independent part of layer 1), transposed
    xy = np.concatenate([x, y], axis=-1)                      # [B, C, 96]
    P = xy @ W1[:XD + YD] + b1                                # [B, C, H]
    PT = np.ascontiguousarray(P.transpose(0, 2, 1))           # [B, H, C]
    pt2 = np.concatenate([PT, PT], axis=2)                    # [B, 128, 512]

    # rT[b, g] = [[rcT(2g); rcT(2g+1)] | [rqT(2g); rqT(2g+1)]]  -> [128, 512]
    rc2 = np.ascontiguousarray(r_c.transpose(0, 1, 3, 2)).reshape(B, C // 2, 128, 256)
    rq2 = np.ascontiguousarray(r_q.transpose(0, 1, 3, 2)).reshape(B, C // 2, 128, 256)
    rt = np.concatenate([rc2, rq2], axis=3)                   # [B, 128, 128, 512]

    # attention maps: transposed, chunked along j, pre-scaled by -ALPHA
    def att_chunks(a):  # [B, i, j] -> [B, 128, 512] = [-0.1*aT ch0 | ch1]
        at = (-0.1 * a.transpose(0, 2, 1)).astype(f32)        # [B, j, i]
        return np.ascontiguousarray(
            at.reshape(B, 2, 128, 256).transpose(0, 2, 1, 3)).reshape(B, 128, 512)

    ac = att_chunks(c_att)
    aq = att_chunks(q_att)

    w1d2 = np.concatenate([W1[XD + YD:], W1[XD + YD:]], axis=0)  # [128, 128]
    i128 = np.eye(128, dtype=f32)

    in_maps = []
    for core in range(NCORES):
        b = core // 2
        g0 = (core % 2) * NG_CORE
        cbig = np.zeros((128, 2368), f32)
        cbig[:, 0:512] = pt2[b]
        cbig[:, 512:1024] = ac[b]
        cbig[:, 1024:1536] = aq[b]
        cbig[:, 1536:1664] = i128
        cbig[:, 1664:1792] = w1d2
        cbig[:, 1792:1920] = W2
        cbig[:, 1920:1984] = W3
        cbig[0, 1984:2112] = 1.0
        cbig[0, 2112:2368] = np.tile(b3, 4)
        in_maps.append({
            "rt": rt[b, g0:g0 + NG_CORE],
            "cbig": cbig,
        })
    return in_maps


def _host_post(results):
    """results[core]["out"] [NG, 128, 512] -> (next_r_c, next_r_q) full."""
    next_r_c = np.empty((B, C, C, E), np.float32)
    next_r_q = np.empty((B, C, C, E), np.float32)
    for core in range(NCORES):
        out = results[core]["out"]                      # [64, 128, 512]
        b = core // 2
        k0 = (core % 2) * 128
        rc = out[:, :, 0:256].reshape(NG_CORE, 2, 64, 256)
        rq = out[:, :, 256:512].reshape(NG_CORE, 2, 64, 256)
        next_r_c[b, k0:k0 + 128] = rc.transpose(0, 1, 3, 2).reshape(128, 256, 64)
        next_r_q[b, k0:k0 + 128] = rq.transpose(0, 1, 3, 2).reshape(128, 256, 64)
    return next_r_c, next_r_q


def kernel(x, y, r_c, r_q, c_att_map, q_att_map, W1, b1, W2, b2, W3, b3,
           _trace=False, _trace_kwargs=None):
    import time
    from concourse.bass_utils import run_bass_kernel_spmd

    t0 = time.time()
    nc = _get_nc()
    t1 = time.time()
    in_maps = _host_prep(x, y, r_c, r_q, c_att_map, q_att_map,
                         W1, b1, W2, b2, W3, b3)
    t2 = time.time()
    res = run_bass_kernel_spmd(
        nc, in_maps, list(range(NCORES)),
        trace=_trace, **(_trace_kwargs or {}))
    t3 = time.time()
    out = _host_post(res.results)
    t4 = time.time()
    kernel.last_result = res
    kernel.timings = {"build": t1 - t0, "prep": t2 - t1, "run": t3 - t2,
                      "post": t4 - t3}
    return out



# revision 5
# speedup vs baseline: 2.1396x; 2.1396x over previous
"""Trainium2 Bass kernel for nn_MetaFunUpdaterLocal (gnn_message_passing).

Math (per meta-batch b, per outer-tile k):
    h    = concat([x[b], y[b], r_c[b,k]], -1)           [C, 160]
    U    = MLP(h)  (160->128 relu ->128 relu ->64)      [C, 64]
    next_r_c[b,k] = r_c[b,k] - 0.1 * c_att[b] @ U
    next_r_q[b,k] = r_q[b,k] - 0.1 * q_att[b] @ U

v2.1 design (vs the fp32 baseline):
  * Everything on the wire is bf16: r tiles, constants, outputs. Halves
    HBM traffic (memory-regime target) and descriptor count.
  * All matmuls bf16 with M=128 stationaries -> FWL (fast weight load)
    removes most LDWEIGHTS overhead vs the fp32r baseline.
  * r tile layout [128, 512] (same as baseline): partitions 0:64 = pair A
    features, 64:128 = pair B; free 0:256 = rc cols, 256:512 = rq cols.
    Layer 1 reads this WITHOUT the baseline's SBUF->SBUF remap: per pair,
    one K=128 matmul with a zero-padded stationary ([W1d;0] for pair A,
    [0;W1d] for pair B) contracts the full partition dim; the other
    pair's rows multiply zeros.
  * P[b] = [x|y]@W1[:96] + b1 (k-independent layer-1 part) is precomputed
    on host and injected into PSUM with a bf16 identity matmul.
  * b3 folded on host into the shipped r tiles (rank-1 correction
    -0.1 * att_rowsum x b3). b2 rides the s2 relu as per-partition bias.
  * Delta matmuls ordered so d0,d2 share the u0 stationary and d1,d3
    share u1 -> 2 stationary loads instead of 4.
  * IO tiles span 2 groups (4 pairs, 256 KiB): loads issue on the SP
    (sync) HWDGE ring, stores on GpSimd SWDGE -- store waits no longer
    head-of-line-block the next loads on the in-order SP stream.
  * Engine balance: Act = both relus, DVE = u copy + final update add.

Sharding: 8 cores, core c handles b = c//2 and a 128-pair slice of the
outer C axis (B x outer-C data parallel per the sharding hint).
"""

import numpy as np

B, C, Q, XD, YD, E, H = 4, 256, 256, 64, 32, 64, 128
NCORES = 8
NG_CORE = 64   # 2-pair groups per core
NIT = 32       # iterations; each handles 2 groups (one 256 KiB IO tile)

_NC_CACHE = {}

# cbig bf16 constant layout (cols):
#   [0:512]     pt2   (P[b].T duplicated for both pairs)
#   [512:1024]  ac    (-0.1 * c_attT, j-chunked)
#   [1024:1536] aq
#   [1536:1664] i128
#   [1664:1792] w1dA  (rows 0:64  = W1[96:160], rows 64:128 = 0)
#   [1792:1920] w1dB  (rows 0:64 = 0, rows 64:128 = W1[96:160])
#   [1920:2048] w2
#   [2048:2112] w3
#   [2112:2114] b2 as 2 bf16 cols bitcast-> fp32 [128,1]
CB = 2114


def _build_nc(nit=NIT):
    import concourse.bass as bass
    import concourse.bacc as bacc
    import concourse.mybir as mybir
    import concourse.tile as tile
    from concourse.bass import _add_dep_helper

    F32 = mybir.dt.float32
    BF16 = mybir.dt.bfloat16
    ADD = mybir.AluOpType.add
    RELU = mybir.ActivationFunctionType.Relu

    nc = bacc.Bacc("TRN2", target_bir_lowering=False, debug=False,
                   num_devices=NCORES)

    rt_d = nc.dram_tensor("rt", [nit, 128, 1024], BF16, kind="ExternalInput")
    cb_d = nc.dram_tensor("cbig", [128, CB], BF16, kind="ExternalInput")
    out_d = nc.dram_tensor("out", [nit, 128, 1024], BF16, kind="ExternalOutput")

    with tile.TileContext(nc) as tc:
        with (
            tc.tile_pool(name="const", bufs=1) as cp,
            tc.tile_pool(name="rt", bufs=4) as rtp,
            tc.tile_pool(name="s1", bufs=3) as s1p,
            tc.tile_pool(name="s2", bufs=3) as s2p,
            tc.tile_pool(name="u", bufs=3) as up,
            tc.tile_pool(name="o", bufs=3) as op,
            tc.tile_pool(name="pz", bufs=4, space="PSUM") as pz,
            tc.tile_pool(name="pu", bufs=2, space="PSUM") as pu,
            tc.tile_pool(name="pd", bufs=2, space="PSUM") as pd,
        ):
            cbig = cp.tile([128, CB], BF16)
            nc.sync.dma_start(cbig[:], cb_d[:, :])
            pt2 = cbig[:, 0:512]
            ac = cbig[:, 512:1024]
            aq = cbig[:, 1024:1536]
            i128 = cbig[:, 1536:1664]
            w1dA = cbig[:, 1664:1792]
            w1dB = cbig[:, 1792:1920]
            w2 = cbig[:, 1920:2048]
            w3 = cbig[:, 2048:2112]
            b2ap = cbig[:, 2112:2114].bitcast(F32)

            def chain(mms):
                for a, b_ in zip(mms[1:], mms):
                    _add_dep_helper(a.ins, b_.ins, sync=False, reason="psum order")

            for it in range(nit):
                rt2 = rtp.tile([128, 1024], BF16)
                nc.sync.dma_start(rt2[:], rt_d[it, :, :])
                o4 = op.tile([128, 1024], BF16)

                for half in range(2):
                    rt = rt2[:, half * 512: half * 512 + 512]
                    o2 = o4[:, half * 512: half * 512 + 512]

                    # ---- layer 1: z1 = P (identity inject) + W1d.T @ rcT
                    # per pair: zero-padded stationary contracts K=128 so the
                    # moving operand stays at partition base 0
                    z1 = pz.tile([128, 512], F32, tag="z")
                    m0 = nc.tensor.matmul(z1[:], i128[:], pt2[:],
                                          start=True, stop=False)
                    m1a = nc.tensor.matmul(z1[:, 0:256], w1dA[:],
                                           rt[:, 0:256], start=False, stop=False)
                    m1b = nc.tensor.matmul(z1[:, 256:512], w1dB[:],
                                           rt[:, 0:256], start=False, stop=True)
                    chain([m0, m1a, m1b])
                    s1 = s1p.tile([128, 512], BF16)
                    nc.scalar.activation(s1[:], z1[:], RELU)

                    # ---- layer 2 (+b2 via Act bias)
                    z2 = pz.tile([128, 512], F32, tag="z")
                    nc.tensor.matmul(z2[:], w2[:], s1[:], start=True, stop=True)
                    s2 = s2p.tile([128, 512], BF16)
                    nc.scalar.activation(s2[:], z2[:], RELU, bias=b2ap)

                    # ---- layer 3: ups cols [A-ch0 | B-ch0 | A-ch1 | B-ch1]
                    ups = pu.tile([128, 256], F32)
                    umms = []
                    for ch in range(2):
                        for p in range(2):
                            mm = nc.tensor.matmul(
                                ups[:, ch * 128 + p * 64: ch * 128 + p * 64 + 64],
                                s2[:, p * 256 + ch * 128: p * 256 + (ch + 1) * 128],
                                w3[:],
                                start=(ch == 0 and p == 0),
                                stop=(ch == 1 and p == 1))
                            umms.append(mm)
                    chain(umms)
                    u = up.tile([128, 256], BF16)
                    nc.vector.tensor_copy(u[:], ups[:])

                    # ---- deltas: dp = [-0.1*Ac@U | -0.1*Aq@U], pair-packed
                    dp = pd.tile([128, 512], F32)
                    d0 = nc.tensor.matmul(dp[:, 0:256], u[:, 0:128],
                                          ac[:, 0:256], start=True, stop=False)
                    d2 = nc.tensor.matmul(dp[:, 256:512], u[:, 0:128],
                                          aq[:, 0:256], start=False, stop=False)
                    d1 = nc.tensor.matmul(dp[:, 0:256], u[:, 128:256],
                                          ac[:, 256:512], start=False, stop=False)
                    d3 = nc.tensor.matmul(dp[:, 256:512], u[:, 128:256],
                                          aq[:, 256:512], start=False, stop=True)
                    chain([d0, d2, d1, d3])

                    # ---- update
                    nc.vector.tensor_tensor(o2[:], rt[:], dp[:], op=ADD)

                # store both halves with one SWDGE dma (keeps the SP ring
                # free of store waits)
                nc.gpsimd.dma_start(out_d[it, :, :], o4[:])

    nc.finalize()
    return nc


def _get_nc(nit=NIT):
    if nit not in _NC_CACHE:
        _NC_CACHE[nit] = _build_nc(nit)
    return _NC_CACHE[nit]


def _to_bf16(x):
    import ml_dtypes
    return np.asarray(x, dtype=ml_dtypes.bfloat16)


def _host_prep(x, y, r_c, r_q, c_att_map, q_att_map, W1, b1, W2, b2, W3, b3):
    """Build per-core input maps. Returns in_maps."""
    import ml_dtypes
    bf = ml_dtypes.bfloat16
    f32 = np.float32
    x = np.asarray(x, f32); y = np.asarray(y, f32)
    r_c = np.asarray(r_c, f32)
    r_q = np.asarray(r_q, f32)
    c_att = np.asarray(c_att_map, f32); q_att = np.asarray(q_att_map, f32)
    W1 = np.asarray(W1, f32); b1 = np.asarray(b1, f32)
    W2 = np.asarray(W2, f32); b2 = np.asarray(b2, f32)
    W3 = np.asarray(W3, f32); b3 = np.asarray(b3, f32)

    # fold b3 into the shipped r tiles: next_r = (r - 0.1*rowsum x b3) - 0.1*att@U'
    if np.any(b3):
        r_c = r_c - 0.1 * c_att.sum(axis=2)[:, None, :, None] * b3
        r_q = r_q - 0.1 * q_att.sum(axis=2)[:, None, :, None] * b3

    # P[b] = [x|y] @ W1[:96] + b1  (k-independent part of layer 1), transposed
    xy = np.concatenate([x, y], axis=-1)                      # [B, C, 96]
    P = xy @ W1[:XD + YD] + b1                                # [B, C, H]
    PT = np.ascontiguousarray(P.transpose(0, 2, 1))           # [B, H, C]
    pt2 = _to_bf16(np.concatenate([PT, PT], axis=2))          # [B, 128, 512]

    # rt[b, g] = [[rcT(2g); rcT(2g+1)] | [rqT(2g); rqT(2g+1)]]  [128, 512]
    rc2 = np.ascontiguousarray(r_c.transpose(0, 1, 3, 2)).reshape(B, C // 2, 128, 256)
    rq2 = np.ascontiguousarray(r_q.transpose(0, 1, 3, 2)).reshape(B, C // 2, 128, 256)
    rt = _to_bf16(np.concatenate([rc2, rq2], axis=3))         # [B, G, 128, 512]

    # attention maps: transposed, chunked along j, pre-scaled by -0.1
    def att_chunks(a):  # [B, i, j] -> [B, 128, 512] = [-0.1*aT ch0 | ch1]
        at = (-0.1 * a.transpose(0, 2, 1)).astype(f32)        # [B, j, i]
        return _to_bf16(np.ascontiguousarray(
            at.reshape(B, 2, 128, 256).transpose(0, 2, 1, 3)).reshape(B, 128, 512))

    acs = att_chunks(c_att)
    aqs = att_chunks(q_att)

    i128 = np.eye(128, dtype=bf)
    w1dA = np.zeros((128, 128), dtype=bf)
    w1dA[:64] = _to_bf16(W1[XD + YD:])
    w1dB = np.zeros((128, 128), dtype=bf)
    w1dB[64:] = _to_bf16(W1[XD + YD:])
    b2_as_bf = np.ascontiguousarray(b2.astype(f32)).view(np.uint16).reshape(128, 2)

    in_maps = []
    for core in range(NCORES):
        b = core // 2
        g0 = (core % 2) * NG_CORE
        cbig = np.zeros((128, CB), dtype=bf)
        cbig[:, 0:512] = pt2[b]
        cbig[:, 512:1024] = acs[b]
        cbig[:, 1024:1536] = aqs[b]
        cbig[:, 1536:1664] = i128
        cbig[:, 1664:1792] = w1dA
        cbig[:, 1792:1920] = w1dB
        cbig[:, 1920:2048] = _to_bf16(W2)
        cbig[:, 2048:2112] = _to_bf16(W3)
        cbig[:, 2112:2114] = b2_as_bf.view(bf)
        # pack 2 consecutive groups side by side on the free dim
        rt_core = rt[b, g0:g0 + NG_CORE].reshape(NIT, 2, 128, 512)
        rt_core = np.ascontiguousarray(
            rt_core.transpose(0, 2, 1, 3)).reshape(NIT, 128, 1024)
        in_maps.append({
            "rt": rt_core,
            "cbig": cbig,
        })
    return in_maps


def _host_post(results):
    """results[core]["out"] [NIT, 128, 1024] bf16 -> (next_r_c, next_r_q)."""
    next_r_c = np.empty((B, C, C, E), np.float32)
    next_r_q = np.empty((B, C, C, E), np.float32)
    for core in range(NCORES):
        out = np.asarray(results[core]["out"]).astype(np.float32)
        out = out.reshape(NIT, 128, 2, 512).transpose(0, 2, 1, 3)
        out = out.reshape(NG_CORE, 128, 512)
        b = core // 2
        k0 = (core % 2) * 128
        rc = out[:, :, 0:256].reshape(NG_CORE, 2, 64, 256)
        rq = out[:, :, 256:512].reshape(NG_CORE, 2, 64, 256)
        next_r_c[b, k0:k0 + 128] = rc.transpose(0, 1, 3, 2).reshape(128, 256, 64)
        next_r_q[b, k0:k0 + 128] = rq.transpose(0, 1, 3, 2).reshape(128, 256, 64)
    return next_r_c, next_r_q


def kernel(x, y, r_c, r_q, c_att_map, q_att_map, W1, b1, W2, b2, W3, b3,
           _trace=False, _trace_kwargs=None):
    import time
    from concourse.bass_utils import run_bass_kernel_spmd

    t0 = time.time()
    nc = _get_nc()
    t1 = time.time()
    in_maps = _host_prep(x, y, r_c, r_q, c_att_map, q_att_map,
                         W1, b1, W2, b2, W3, b3)
    t2 = time.time()
    res = run_bass_kernel_spmd(
        nc, in_maps, list(range(NCORES)),
        trace=_trace, **(_trace_kwargs or {}))
    t3 = time.time()
    out = _host_post(res.results)
    t4 = time.time()
    kernel.last_result = res
    kernel.timings = {"build": t1 - t0, "prep": t2 - t1, "run": t3 - t2,
                      "post": t4 - t3}
    return out


# revision 12
# speedup vs baseline: 2.8321x; 1.3237x over previous
"""Trainium2 Bass kernel for nn_MetaFunUpdaterLocal (gnn_message_passing).

Math (per meta-batch b, per outer-tile k):
    h    = concat([x[b], y[b], r_c[b,k]], -1)           [C, 160]
    U    = MLP(h)  (160->128 relu ->128 relu ->64)      [C, 64]
    next_r_c[b,k] = r_c[b,k] - 0.1 * c_att[b] @ U
    next_r_q[b,k] = r_q[b,k] - 0.1 * q_att[b] @ U

v2.1 design (vs the fp32 baseline):
  * Everything on the wire is bf16: r tiles, constants, outputs. Halves
    HBM traffic (memory-regime target) and descriptor count.
  * All matmuls bf16 with M=128 stationaries -> FWL (fast weight load)
    removes most LDWEIGHTS overhead vs the fp32r baseline.
  * r tile layout [128, 512] (same as baseline): partitions 0:64 = pair A
    features, 64:128 = pair B; free 0:256 = rc cols, 256:512 = rq cols.
    Layer 1 reads this WITHOUT the baseline's SBUF->SBUF remap: per pair,
    one K=128 matmul with a zero-padded stationary ([W1d;0] for pair A,
    [0;W1d] for pair B) contracts the full partition dim; the other
    pair's rows multiply zeros.
  * P[b] = [x|y]@W1[:96] + b1 (k-independent layer-1 part) is precomputed
    on host and injected into PSUM with a bf16 identity matmul.
  * b3 folded on host into the shipped r tiles (rank-1 correction
    -0.1 * att_rowsum x b3). b2 rides the s2 relu as per-partition bias.
  * Delta matmuls ordered so d0,d2 share the u0 stationary and d1,d3
    share u1 -> 2 stationary loads instead of 4.
  * IO tiles span 2 groups (4 pairs, 256 KiB): loads issue on the SP
    (sync) HWDGE ring, stores on GpSimd SWDGE -- store waits no longer
    head-of-line-block the next loads on the in-order SP stream.
  * Engine balance: Act = both relus, DVE = u copy + final update add.

Sharding: 8 cores, core c handles b = c//2 and a 128-pair slice of the
outer C axis (B x outer-C data parallel per the sharding hint).
"""

import numpy as np

B, C, Q, XD, YD, E, H = 4, 256, 256, 64, 32, 64, 128
NCORES = 8
NG_CORE = 64   # 2-pair groups per core
NIT = 32       # iterations; each handles 2 groups (one 256 KiB IO tile)

_NC_CACHE = {}

# cbig bf16 constant layout (cols):
#   [0:512]     pt2   (P[b].T duplicated for both pairs)
#   [512:1024]  ac    (-0.1 * c_attT, j-chunked)
#   [1024:1536] aq
#   [1536:1664] i128
#   [1664:1792] w1dA  (rows 0:64  = W1[96:160], rows 64:128 = 0)
#   [1792:1920] w1dB  (rows 0:64 = 0, rows 64:128 = W1[96:160])
#   [1920:2048] w2
#   [2048:2112] w3
#   [2112:2114] b2 as 2 bf16 cols bitcast-> fp32 [128,1]
CB = 2114


def _build_nc(nit=NIT):
    import concourse.bass as bass
    import concourse.bacc as bacc
    import concourse.mybir as mybir
    import concourse.tile as tile
    from concourse.bass import _add_dep_helper

    F32 = mybir.dt.float32
    BF16 = mybir.dt.bfloat16
    FP8 = mybir.dt.float8e4
    ADD = mybir.AluOpType.add
    RELU = mybir.ActivationFunctionType.Relu
    DR = mybir.MatmulPerfMode.DoubleRow

    nc = bacc.Bacc("TRN2", target_bir_lowering=False, debug=False,
                   num_devices=NCORES)

    rt_d = nc.dram_tensor("rt", [nit, 128, 1024], BF16, kind="ExternalInput")
    cb_d = nc.dram_tensor("cbig", [128, CB], BF16, kind="ExternalInput")
    out_d = nc.dram_tensor("out", [nit, 128, 1024], BF16, kind="ExternalOutput")

    with tile.TileContext(nc) as tc:
        with (
            tc.tile_pool(name="const", bufs=1) as cp,
            tc.tile_pool(name="rt", bufs=4) as rtp,
            tc.tile_pool(name="s1", bufs=3) as s1p,
            tc.tile_pool(name="s2", bufs=3) as s2p,
            tc.tile_pool(name="u", bufs=3) as up,
            tc.tile_pool(name="o", bufs=3) as op,
            tc.tile_pool(name="pz", bufs=4, space="PSUM") as pz,
            tc.tile_pool(name="pud", bufs=4, space="PSUM") as pud,
        ):
            cbig = cp.tile([128, CB], BF16)
            nc.sync.dma_start(cbig[:], cb_d[:, :])
            pt2 = cbig[:, 0:512]
            ac = cbig[:, 512:1024]
            aq = cbig[:, 1024:1536]
            i128 = cbig[:, 1536:1664]
            w1dA = cbig[:, 1664:1792]
            w1dB = cbig[:, 1792:1920]
            w2 = cbig[:, 1920:2048]
            w3 = cbig[:, 2048:2112]
            b2ap = cbig[:, 2112:2114].bitcast(F32)

            def chain(mms):
                for a, b_ in zip(mms[1:], mms):
                    _add_dep_helper(a.ins, b_.ins, sync=False, reason="psum order")

            for it in range(nit):
                rt2 = rtp.tile([128, 1024], BF16)
                nc.sync.dma_start(rt2[:], rt_d[it, :, :])
                o4 = op.tile([128, 1024], BF16)

                for half in range(2):
                    rt = rt2[:, half * 512: half * 512 + 512]
                    o2 = o4[:, half * 512: half * 512 + 512]

                    # ---- layer 1: z = P (identity inject) + W1d.T @ rcT
                    # per pair: zero-padded stationary contracts K=128 so the
                    # moving operand stays at partition base 0
                    z = pz.tile([128, 512], F32)
                    m0 = nc.tensor.matmul(z[:], i128[:], pt2[:],
                                          start=True, stop=False)
                    m1a = nc.tensor.matmul(z[:, 0:256], w1dA[:],
                                           rt[:, 0:256], start=False, stop=False)
                    m1b = nc.tensor.matmul(z[:, 256:512], w1dB[:],
                                           rt[:, 0:256], start=False, stop=True)
                    chain([m0, m1a, m1b])
                    s1 = s1p.tile([128, 512], BF16)
                    nc.scalar.activation(s1[:], z[:], RELU)

                    # ---- layer 2 reuses z's PSUM bank (z2 already depends on
                    # relu1 through s1, so the WAR reuse costs nothing)
                    nc.tensor.matmul(z[:], w2[:], s1[:], start=True, stop=True)
                    s2 = s2p.tile([128, 512], BF16)
                    nc.scalar.activation(s2[:], z[:], RELU, bias=b2ap)

                    # ---- layer 3 into ud[:, 0:256]: [A-ch0 | B-ch0 | A-ch1 | B-ch1]
                    ud = pud.tile([128, 512], F32)
                    umms = []
                    for ch in range(2):
                        for p in range(2):
                            mm = nc.tensor.matmul(
                                ud[:, ch * 128 + p * 64: ch * 128 + p * 64 + 64],
                                s2[:, p * 256 + ch * 128: p * 256 + (ch + 1) * 128],
                                w3[:],
                                start=(ch == 0 and p == 0),
                                stop=(ch == 1 and p == 1))
                            umms.append(mm)
                    chain(umms)
                    u = up.tile([128, 256], BF16)
                    nc.vector.tensor_copy(u[:], ud[:, 0:256])

                    # ---- deltas overwrite ud (they depend on the u cast);
                    # d0,d2 share stationary u0; d1,d3 share u1
                    d0 = nc.tensor.matmul(ud[:, 0:256], u[:, 0:128],
                                          ac[:, 0:256], start=True, stop=False)
                    d2 = nc.tensor.matmul(ud[:, 256:512], u[:, 0:128],
                                          aq[:, 0:256], start=False, stop=False)
                    d1 = nc.tensor.matmul(ud[:, 0:256], u[:, 128:256],
                                          ac[:, 256:512], start=False, stop=False)
                    d3 = nc.tensor.matmul(ud[:, 256:512], u[:, 128:256],
                                          aq[:, 256:512], start=False, stop=True)
                    chain([d0, d2, d1, d3])

                    # ---- update
                    nc.vector.tensor_tensor(o2[:], rt[:], ud[:], op=ADD)

                # store both halves with one SWDGE dma (keeps the SP ring
                # free of store waits)
                nc.gpsimd.dma_start(out_d[it, :, :], o4[:])

    nc.finalize()
    return nc


def _get_nc(nit=NIT):
    if nit not in _NC_CACHE:
        _NC_CACHE[nit] = _build_nc(nit)
    return _NC_CACHE[nit]


def _to_bf16(x):
    import ml_dtypes
    return np.asarray(x, dtype=ml_dtypes.bfloat16)


def _host_prep(x, y, r_c, r_q, c_att_map, q_att_map, W1, b1, W2, b2, W3, b3):
    """Build per-core input maps. Returns in_maps."""
    import ml_dtypes
    bf = ml_dtypes.bfloat16
    f32 = np.float32
    x = np.asarray(x, f32); y = np.asarray(y, f32)
    r_c = np.asarray(r_c, f32)
    r_q = np.asarray(r_q, f32)
    c_att = np.asarray(c_att_map, f32); q_att = np.asarray(q_att_map, f32)
    W1 = np.asarray(W1, f32); b1 = np.asarray(b1, f32)
    W2 = np.asarray(W2, f32); b2 = np.asarray(b2, f32)
    W3 = np.asarray(W3, f32); b3 = np.asarray(b3, f32)

    # fold b3 into the shipped r tiles: next_r = (r - 0.1*rowsum x b3) - 0.1*att@U'
    if np.any(b3):
        r_c = r_c - 0.1 * c_att.sum(axis=2)[:, None, :, None] * b3
        r_q = r_q - 0.1 * q_att.sum(axis=2)[:, None, :, None] * b3

    # P[b] = [x|y] @ W1[:96] + b1  (k-independent part of layer 1), transposed
    xy = np.concatenate([x, y], axis=-1)                      # [B, C, 96]
    P = xy @ W1[:XD + YD] + b1                                # [B, C, H]
    PT = np.ascontiguousarray(P.transpose(0, 2, 1))           # [B, H, C]
    pt2 = _to_bf16(np.concatenate([PT, PT], axis=2))          # [B, 128, 512]

    # rt[b, g] = [[rcT(2g); rcT(2g+1)] | [rqT(2g); rqT(2g+1)]]  [128, 512]
    rc2 = np.ascontiguousarray(r_c.transpose(0, 1, 3, 2)).reshape(B, C // 2, 128, 256)
    rq2 = np.ascontiguousarray(r_q.transpose(0, 1, 3, 2)).reshape(B, C // 2, 128, 256)
    rt = _to_bf16(np.concatenate([rc2, rq2], axis=3))         # [B, G, 128, 512]

    # attention maps: transposed, chunked along j, pre-scaled by -0.1
    def att_chunks(a):  # [B, i, j] -> [B, 128, 512] = [-0.1*aT ch0 | ch1]
        at = (-0.1 * a.transpose(0, 2, 1)).astype(f32)        # [B, j, i]
        return _to_bf16(np.ascontiguousarray(
            at.reshape(B, 2, 128, 256).transpose(0, 2, 1, 3)).reshape(B, 128, 512))

    acs = att_chunks(c_att)
    aqs = att_chunks(q_att)

    i128 = np.eye(128, dtype=bf)
    w1dA = np.zeros((128, 128), dtype=bf)
    w1dA[:64] = _to_bf16(W1[XD + YD:])
    w1dB = np.zeros((128, 128), dtype=bf)
    w1dB[64:] = _to_bf16(W1[XD + YD:])
    b2_as_bf = np.ascontiguousarray(b2.astype(f32)).view(np.uint16).reshape(128, 2)

    in_maps = []
    for core in range(NCORES):
        b = core // 2
        g0 = (core % 2) * NG_CORE
        cbig = np.zeros((128, CB), dtype=bf)
        cbig[:, 0:512] = pt2[b]
        cbig[:, 512:1024] = acs[b]
        cbig[:, 1024:1536] = aqs[b]
        cbig[:, 1536:1664] = i128
        cbig[:, 1664:1792] = w1dA
        cbig[:, 1792:1920] = w1dB
        cbig[:, 1920:2048] = _to_bf16(W2)
        cbig[:, 2048:2112] = _to_bf16(W3)
        cbig[:, 2112:2114] = b2_as_bf.view(bf)
        # pack 2 consecutive groups side by side on the free dim
        rt_core = rt[b, g0:g0 + NG_CORE].reshape(NIT, 2, 128, 512)
        rt_core = np.ascontiguousarray(
            rt_core.transpose(0, 2, 1, 3)).reshape(NIT, 128, 1024)
        in_maps.append({
            "rt": rt_core,
            "cbig": cbig,
        })
    return in_maps


def _host_post(results):
    """results[core]["out"] [NIT, 128, 1024] bf16 -> (next_r_c, next_r_q)."""
    next_r_c = np.empty((B, C, C, E), np.float32)
    next_r_q = np.empty((B, C, C, E), np.float32)
    for core in range(NCORES):
        out = np.asarray(results[core]["out"]).astype(np.float32)
        out = out.reshape(NIT, 128, 2, 512).transpose(0, 2, 1, 3)
        out = out.reshape(NG_CORE, 128, 512)
        b = core // 2
        k0 = (core % 2) * 128
        rc = out[:, :, 0:256].reshape(NG_CORE, 2, 64, 256)
        rq = out[:, :, 256:512].reshape(NG_CORE, 2, 64, 256)
        next_r_c[b, k0:k0 + 128] = rc.transpose(0, 1, 3, 2).reshape(128, 256, 64)
        next_r_q[b, k0:k0 + 128] = rq.transpose(0, 1, 3, 2).reshape(128, 256, 64)
    return next_r_c, next_r_q


def kernel(x, y, r_c, r_q, c_att_map, q_att_map, W1, b1, W2, b2, W3, b3,
           _trace=False, _trace_kwargs=None):
    import time
    from concourse.bass_utils import run_bass_kernel_spmd

    t0 = time.time()
    nc = _get_nc()
    t1 = time.time()
    in_maps = _host_prep(x, y, r_c, r_q, c_att_map, q_att_map,
                         W1, b1, W2, b2, W3, b3)
    t2 = time.time()
    res = run_bass_kernel_spmd(
        nc, in_maps, list(range(NCORES)),
        trace=_trace, **(_trace_kwargs or {}))
    t3 = time.time()
    out = _host_post(res.results)
    t4 = time.time()
    kernel.last_result = res
    kernel.timings = {"build": t1 - t0, "prep": t2 - t1, "run": t3 - t2,
                      "post": t4 - t3}
    return out


# revision 16
# speedup vs baseline: 3.0294x; 1.0697x over previous
"""Trainium2 Bass kernel for nn_MetaFunUpdaterLocal (gnn_message_passing).

Math (per meta-batch b, per outer-tile k):
    h    = concat([x[b], y[b], r_c[b,k]], -1)           [C, 160]
    U    = MLP(h)  (160->128 relu ->128 relu ->64)      [C, 64]
    next_r_c[b,k] = r_c[b,k] - 0.1 * c_att[b] @ U
    next_r_q[b,k] = r_q[b,k] - 0.1 * q_att[b] @ U

v2.1 design (vs the fp32 baseline):
  * Everything on the wire is bf16: r tiles, constants, outputs. Halves
    HBM traffic (memory-regime target) and descriptor count.
  * All matmuls bf16 with M=128 stationaries -> FWL (fast weight load)
    removes most LDWEIGHTS overhead vs the fp32r baseline.
  * r tile layout [128, 512] (same as baseline): partitions 0:64 = pair A
    features, 64:128 = pair B; free 0:256 = rc cols, 256:512 = rq cols.
    Layer 1 reads this WITHOUT the baseline's SBUF->SBUF remap: per pair,
    one K=128 matmul with a zero-padded stationary ([W1d;0] for pair A,
    [0;W1d] for pair B) contracts the full partition dim; the other
    pair's rows multiply zeros.
  * P[b] = [x|y]@W1[:96] + b1 (k-independent layer-1 part) is precomputed
    on host and injected into PSUM with a bf16 identity matmul.
  * b3 folded on host into the shipped r tiles (rank-1 correction
    -0.1 * att_rowsum x b3). b2 rides the s2 relu as per-partition bias.
  * Delta matmuls ordered so d0,d2 share the u0 stationary and d1,d3
    share u1 -> 2 stationary loads instead of 4.
  * IO tiles span 2 groups (4 pairs, 256 KiB): loads issue on the SP
    (sync) HWDGE ring, stores on GpSimd SWDGE -- store waits no longer
    head-of-line-block the next loads on the in-order SP stream.
  * Engine balance: Act = both relus, DVE = u copy + final update add.

Sharding: 8 cores, core c handles b = c//2 and a 128-pair slice of the
outer C axis (B x outer-C data parallel per the sharding hint).
"""

import numpy as np

B, C, Q, XD, YD, E, H = 4, 256, 256, 64, 32, 64, 128
NCORES = 8
NG_CORE = 64   # 2-pair groups per core
NIT = 32       # iterations; each handles 2 groups (one 256 KiB IO tile)

_NC_CACHE = {}

# cbig bf16 constant layout (cols):
#   [0:512]     pt2   (P[b].T duplicated for both pairs)
#   [512:1024]  ac    (-0.1 * c_attT, j-chunked)
#   [1024:1536] aq
#   [1536:1664] i128
#   [1664:1792] w1dA  (rows 0:64  = W1[96:160], rows 64:128 = 0)
#   [1792:1920] w1dB  (rows 0:64 = 0, rows 64:128 = W1[96:160])
#   [1920:2048] w2
#   [2048:2112] w3
#   [2112:2114] b2 as 2 bf16 cols bitcast-> fp32 [128,1]
CB = 2114


def _build_nc(nit=NIT):
    import concourse.bass as bass
    import concourse.bacc as bacc
    import concourse.mybir as mybir
    import concourse.tile as tile
    from concourse.bass import _add_dep_helper

    F32 = mybir.dt.float32
    BF16 = mybir.dt.bfloat16
    FP8 = mybir.dt.float8e4
    ADD = mybir.AluOpType.add
    RELU = mybir.ActivationFunctionType.Relu
    DR = mybir.MatmulPerfMode.DoubleRow

    nc = bacc.Bacc("TRN2", target_bir_lowering=False, debug=False,
                   num_devices=NCORES)

    rt_d = nc.dram_tensor("rt", [nit, 128, 1024], BF16, kind="ExternalInput")
    cb_d = nc.dram_tensor("cbig", [128, CB], BF16, kind="ExternalInput")
    out_d = nc.dram_tensor("out", [nit, 128, 1024], BF16, kind="ExternalOutput")

    with tile.TileContext(nc) as tc:
        with (
            tc.tile_pool(name="const", bufs=1) as cp,
            tc.tile_pool(name="rt", bufs=6) as rtp,
            tc.tile_pool(name="s1", bufs=6) as s1p,
            tc.tile_pool(name="s2", bufs=6) as s2p,
            tc.tile_pool(name="u", bufs=6) as up,
            tc.tile_pool(name="o", bufs=4) as op,
            tc.tile_pool(name="pzu", bufs=8, space="PSUM") as pzu,
        ):
            cbig = cp.tile([128, CB], BF16)
            nc.sync.dma_start(cbig[:], cb_d[:, :])
            pt2 = cbig[:, 0:512]
            ac = cbig[:, 512:1024]
            aq = cbig[:, 1024:1536]
            i128 = cbig[:, 1536:1664]
            w1dA = cbig[:, 1664:1792]
            w1dB = cbig[:, 1792:1920]
            w2 = cbig[:, 1920:2048]
            w3 = cbig[:, 2048:2112]
            b2ap = cbig[:, 2112:2114].bitcast(F32)

            def chain(mms):
                for a, b_ in zip(mms[1:], mms):
                    _add_dep_helper(a.ins, b_.ins, sync=False, reason="psum order")

            for it in range(nit):
                rt2 = rtp.tile([128, 1024], BF16)
                nc.sync.dma_start(rt2[:], rt_d[it, :, :])
                o4 = op.tile([128, 1024], BF16)

                for half in range(2):
                    rt = rt2[:, half * 512: half * 512 + 512]
                    o2 = o4[:, half * 512: half * 512 + 512]

                    # ---- one PSUM tile serves z1 -> z2 -> ups -> dp: every
                    # reuse is write-after-read already ordered by dataflow,
                    # so 8 bufs = 8 halves in flight
                    z = pzu.tile([128, 512], F32)
                    m0 = nc.tensor.matmul(z[:], i128[:], pt2[:],
                                          start=True, stop=False)
                    m1a = nc.tensor.matmul(z[:, 0:256], w1dA[:],
                                           rt[:, 0:256], start=False, stop=False)
                    m1b = nc.tensor.matmul(z[:, 256:512], w1dB[:],
                                           rt[:, 0:256], start=False, stop=True)
                    chain([m0, m1a, m1b])
                    s1 = s1p.tile([128, 512], BF16)
                    nc.scalar.activation(s1[:], z[:], RELU)

                    # ---- layer 2 reuses z's PSUM bank (z2 already depends on
                    # relu1 through s1, so the WAR reuse costs nothing)
                    nc.tensor.matmul(z[:], w2[:], s1[:], start=True, stop=True)
                    s2 = s2p.tile([128, 512], BF16)
                    nc.scalar.activation(s2[:], z[:], RELU, bias=b2ap)

                    # ---- layer 3 into ud[:, 0:256]: [A-ch0 | B-ch0 | A-ch1 | B-ch1]
                    ud = z
                    umms = []
                    for ch in range(2):
                        for p in range(2):
                            mm = nc.tensor.matmul(
                                ud[:, ch * 128 + p * 64: ch * 128 + p * 64 + 64],
                                s2[:, p * 256 + ch * 128: p * 256 + (ch + 1) * 128],
                                w3[:],
                                start=(ch == 0 and p == 0),
                                stop=(ch == 1 and p == 1))
                            umms.append(mm)
                    chain(umms)
                    u = up.tile([128, 256], BF16)
                    nc.vector.tensor_copy(u[:], ud[:, 0:256])

                    # ---- deltas overwrite ud (they depend on the u cast);
                    # d0,d2 share stationary u0; d1,d3 share u1
                    d0 = nc.tensor.matmul(ud[:, 0:256], u[:, 0:128],
                                          ac[:, 0:256], start=True, stop=False)
                    d2 = nc.tensor.matmul(ud[:, 256:512], u[:, 0:128],
                                          aq[:, 0:256], start=False, stop=False)
                    d1 = nc.tensor.matmul(ud[:, 0:256], u[:, 128:256],
                                          ac[:, 256:512], start=False, stop=False)
                    d3 = nc.tensor.matmul(ud[:, 256:512], u[:, 128:256],
                                          aq[:, 256:512], start=False, stop=True)
                    chain([d0, d2, d1, d3])

                    # ---- update
                    nc.vector.tensor_tensor(o2[:], rt[:], ud[:], op=ADD)

                # store both halves with one SWDGE dma (keeps the SP ring
                # free of store waits)
                nc.gpsimd.dma_start(out_d[it, :, :], o4[:])

    nc.finalize()
    return nc


def _get_nc(nit=NIT):
    if nit not in _NC_CACHE:
        _NC_CACHE[nit] = _build_nc(nit)
    return _NC_CACHE[nit]


def _to_bf16(x):
    import ml_dtypes
    return np.asarray(x, dtype=ml_dtypes.bfloat16)


def _host_prep(x, y, r_c, r_q, c_att_map, q_att_map, W1, b1, W2, b2, W3, b3):
    """Build per-core input maps. Returns in_maps."""
    import ml_dtypes
    bf = ml_dtypes.bfloat16
    f32 = np.float32
    x = np.asarray(x, f32); y = np.asarray(y, f32)
    r_c = np.asarray(r_c, f32)
    r_q = np.asarray(r_q, f32)
    c_att = np.asarray(c_att_map, f32); q_att = np.asarray(q_att_map, f32)
    W1 = np.asarray(W1, f32); b1 = np.asarray(b1, f32)
    W2 = np.asarray(W2, f32); b2 = np.asarray(b2, f32)
    W3 = np.asarray(W3, f32); b3 = np.asarray(b3, f32)

    # fold b3 into the shipped r tiles: next_r = (r - 0.1*rowsum x b3) - 0.1*att@U'
    if np.any(b3):
        r_c = r_c - 0.1 * c_att.sum(axis=2)[:, None, :, None] * b3
        r_q = r_q - 0.1 * q_att.sum(axis=2)[:, None, :, None] * b3

    # P[b] = [x|y] @ W1[:96] + b1  (k-independent part of layer 1), transposed
    xy = np.concatenate([x, y], axis=-1)                      # [B, C, 96]
    P = xy @ W1[:XD + YD] + b1                                # [B, C, H]
    PT = np.ascontiguousarray(P.transpose(0, 2, 1))           # [B, H, C]
    pt2 = _to_bf16(np.concatenate([PT, PT], axis=2))          # [B, 128, 512]

    # rt[b, g] = [[rcT(2g); rcT(2g+1)] | [rqT(2g); rqT(2g+1)]]  [128, 512]
    rc2 = np.ascontiguousarray(r_c.transpose(0, 1, 3, 2)).reshape(B, C // 2, 128, 256)
    rq2 = np.ascontiguousarray(r_q.transpose(0, 1, 3, 2)).reshape(B, C // 2, 128, 256)
    rt = _to_bf16(np.concatenate([rc2, rq2], axis=3))         # [B, G, 128, 512]

    # attention maps: transposed, chunked along j, pre-scaled by -0.1
    def att_chunks(a):  # [B, i, j] -> [B, 128, 512] = [-0.1*aT ch0 | ch1]
        at = (-0.1 * a.transpose(0, 2, 1)).astype(f32)        # [B, j, i]
        return _to_bf16(np.ascontiguousarray(
            at.reshape(B, 2, 128, 256).transpose(0, 2, 1, 3)).reshape(B, 128, 512))

    acs = att_chunks(c_att)
    aqs = att_chunks(q_att)

    i128 = np.eye(128, dtype=bf)
    w1dA = np.zeros((128, 128), dtype=bf)
    w1dA[:64] = _to_bf16(W1[XD + YD:])
    w1dB = np.zeros((128, 128), dtype=bf)
    w1dB[64:] = _to_bf16(W1[XD + YD:])
    b2_as_bf = np.ascontiguousarray(b2.astype(f32)).view(np.uint16).reshape(128, 2)

    in_maps = []
    for core in range(NCORES):
        b = core // 2
        g0 = (core % 2) * NG_CORE
        cbig = np.zeros((128, CB), dtype=bf)
        cbig[:, 0:512] = pt2[b]
        cbig[:, 512:1024] = acs[b]
        cbig[:, 1024:1536] = aqs[b]
        cbig[:, 1536:1664] = i128
        cbig[:, 1664:1792] = w1dA
        cbig[:, 1792:1920] = w1dB
        cbig[:, 1920:2048] = _to_bf16(W2)
        cbig[:, 2048:2112] = _to_bf16(W3)
        cbig[:, 2112:2114] = b2_as_bf.view(bf)
        # pack 2 consecutive groups side by side on the free dim
        rt_core = rt[b, g0:g0 + NG_CORE].reshape(NIT, 2, 128, 512)
        rt_core = np.ascontiguousarray(
            rt_core.transpose(0, 2, 1, 3)).reshape(NIT, 128, 1024)
        in_maps.append({
            "rt": rt_core,
            "cbig": cbig,
        })
    return in_maps


def _host_post(results):
    """results[core]["out"] [NIT, 128, 1024] bf16 -> (next_r_c, next_r_q)."""
    next_r_c = np.empty((B, C, C, E), np.float32)
    next_r_q = np.empty((B, C, C, E), np.float32)
    for core in range(NCORES):
        out = np.asarray(results[core]["out"]).astype(np.float32)
        out = out.reshape(NIT, 128, 2, 512).transpose(0, 2, 1, 3)
        out = out.reshape(NG_CORE, 128, 512)
        b = core // 2
        k0 = (core % 2) * 128
        rc = out[:, :, 0:256].reshape(NG_CORE, 2, 64, 256)
        rq = out[:, :, 256:512].reshape(NG_CORE, 2, 64, 256)
        next_r_c[b, k0:k0 + 128] = rc.transpose(0, 1, 3, 2).reshape(128, 256, 64)
        next_r_q[b, k0:k0 + 128] = rq.transpose(0, 1, 3, 2).reshape(128, 256, 64)
    return next_r_c, next_r_q


def kernel(x, y, r_c, r_q, c_att_map, q_att_map, W1, b1, W2, b2, W3, b3,
           _trace=False, _trace_kwargs=None):
    import time
    from concourse.bass_utils import run_bass_kernel_spmd

    t0 = time.time()
    nc = _get_nc()
    t1 = time.time()
    in_maps = _host_prep(x, y, r_c, r_q, c_att_map, q_att_map,
                         W1, b1, W2, b2, W3, b3)
    t2 = time.time()
    res = run_bass_kernel_spmd(
        nc, in_maps, list(range(NCORES)),
        trace=_trace, **(_trace_kwargs or {}))
    t3 = time.time()
    out = _host_post(res.results)
    t4 = time.time()
    kernel.last_result = res
    kernel.timings = {"build": t1 - t0, "prep": t2 - t1, "run": t3 - t2,
                      "post": t4 - t3}
    return out
